# revision 14
# baseline (speedup 1.0000x reference)
"""Longformer classifier on 8 TRN2 NeuronCores.

Sharding: 2 batch groups x 4-way sequence parallel (1024 tokens/core).
Activations live transposed in SBUF ([hid, tok]); matmul inputs bf16,
accumulation/softmax/LN in fp32. Per layer one AllGather exchanges K/V
halo blocks + global-token data within each 4-core batch group (halo
slot selection via per-core one-hot coefficients, keeping the program
SPMD-static), and a second tiny AllGather combines global-attention
softmax partials.
"""
import contextlib
import os
import numpy as np
import ml_dtypes

import concourse.bass as bass
import concourse.mybir as mybir
import concourse.tile as tile
import concourse.bacc as bacc
from concourse.bass_utils import run_bass_kernel_spmd

F32 = mybir.dt.float32
BF16 = mybir.dt.bfloat16
AF = mybir.ActivationFunctionType
ALU = mybir.AluOpType

HID, H, D, NL_FULL, FFN, W = 768, 12, 64, 12, 3072, 256
S, B, NCLS = 4096, 2, 43
CH = 1024               # tokens per core
KT = HID // 128         # 6 hid subtiles
TT = CH // 128          # 8 token subtiles per core
NQB = CH // W           # 4 query blocks of 256
GSUB = 12               # global window subtile index
WLEN = 13 * 128         # kwin cols: 256 halo | 1024 own | 256 halo | 128 global
RG = [[0, 1, 2, 3], [4, 5, 6, 7]]
NLAYERS = int(os.environ.get("KERNEL_NLAYERS", str(NL_FULL)))
DEBUG_DUMPS = bool(int(os.environ.get("KERNEL_DEBUG", "0")))

BF = ml_dtypes.bfloat16

# ag1 region offsets (bf16 elements)
_KF = 0                      # K first block  [768, 256]
_KL = _KF + HID * W          # K last block   [768, 256]
_VF = _KL + HID * W          # V first block  [256, 768]
_VL = _VF + W * HID          # V last block   [256, 768]
_QG = _VL + W * HID          # qg             [768]
AG1N = _QG + HID




_PERS = {}


def _tctile(tc, shape, dtype, name):
    pool = _PERS["pool"]
    return pool.tile(list(shape), dtype, tag=name, name=name)

def build_program(nl=NLAYERS):
    nc = bacc.Bacc()
    dp = nc.declare_dram_parameter
    x0T_d = dp("x0T", [KT, 128, CH], F32, isOutput=False)
    wq_d = dp("wq", [nl, HID, HID], BF16, isOutput=False)
    wk_d = dp("wk", [nl, HID, HID], BF16, isOutput=False)
    wv_d = dp("wv", [nl, HID, HID], BF16, isOutput=False)
    wkg_d = dp("wkg", [nl, HID, HID], BF16, isOutput=False)
    wvg_d = dp("wvg", [nl, HID, HID], BF16, isOutput=False)
    wqg_d = dp("wqg", [nl, HID, HID], BF16, isOutput=False)
    wo_d = dp("wo", [nl, HID, HID], BF16, isOutput=False)
    w1_d = dp("w1", [nl, HID, FFN], BF16, isOutput=False)
    w2_d = dp("w2", [nl, FFN, HID], BF16, isOutput=False)
    # order: bq, bk, bkg, bqg, bo, ln1g, ln1b, ln2g, ln2b, b2
    bcols_d = dp("bcols", [nl, 10, HID], F32, isOutput=False)
    b1_d = dp("b1", [nl, FFN], F32, isOutput=False)
    bvb_d = dp("bvb", [nl, 128, HID], BF16, isOutput=False)
    bvgb_d = dp("bvgb", [nl, 128, HID], BF16, isOutput=False)
    embg_d = dp("embln", [2, HID], F32, isOutput=False)
    masks_d = dp("masks", [NQB, 7, 128, W], BF16, isOutput=False)
    padm_d = dp("padm", [H, CH], BF16, isOutput=False)
    iso_d = dp("iso", [128, 1], mybir.dt.uint8, isOutput=False)
    selL_d = dp("selL", [128, 4], F32, isOutput=False)
    selR_d = dp("selR", [128, 4], F32, isOutput=False)
    ident_d = dp("ident", [128, 128], BF16, isOutput=False)
    eexp_d = dp("eexp", [KT, H, 128], BF16, isOutput=False)
    hw1_d = dp("hw1", [HID, 512], BF16, isOutput=False)
    hb1_d = dp("hb1", [512], F32, isOutput=False)
    hw2_d = dp("hw2", [512, NCLS], BF16, isOutput=False)
    hb2_d = dp("hb2", [NCLS], F32, isOutput=False)
    logits_d = dp("logits", [NCLS, 1], F32, isOutput=True)
    xfin_d = dp("xfin", [KT, 128, CH], F32, isOutput=True)
    dbg_d = dp("dbg", [nl, 2, KT, 128, CH], F32, isOutput=True) if DEBUG_DUMPS else None

    ag1_in = [nc.dram_tensor(f"ag1_in_{l}", [AG1N], BF16) for l in range(nl)]
    ag1_out = [nc.dram_tensor(f"ag1_out_{l}", [4, AG1N], BF16) for l in range(nl)]
    ag2_in = [nc.dram_tensor(f"ag2_in_{l}", [H, 65], F32) for l in range(nl)]
    ag2_out = [nc.dram_tensor(f"ag2_out_{l}", [4, H, 65], F32) for l in range(nl)]

    def _act(out, in_, func=AF.Identity, bias=0.0, scale=1.0):
        nc.scalar.activation(out, in_, func, bias=bias, scale=scale)

    with tile.TileContext(nc) as tc:
        _pers_ctx = contextlib.ExitStack()
        _PERS["pool"] = _pers_ctx.enter_context(
            tc.tile_pool(name="pers", bufs=1))
        xT = [_tctile(tc, [128, CH], F32, name=f"xT{o}") for o in range(KT)]
        xbf = [_tctile(tc, [128, CH], BF16, name=f"xbf{o}") for o in range(KT)]
        qT = [_tctile(tc, [128, CH], BF16, name=f"qT{o}") for o in range(KT)]
        kwin = [_tctile(tc, [128, WLEN], BF16, name=f"kwin{o}") for o in range(KT)]
        vwin = [_tctile(tc, [128, H, 65], BF16, name=f"vwin{t}") for t in range(13)]
        attnT = [_tctile(tc, [128, CH], BF16, name=f"attnT{o}") for o in range(KT)]
        sums = _tctile(tc, [H, CH], F32, name="sums")
        rsums_bf = _tctile(tc, [H, CH], BF16, name="rsums_bf")
        pgbf = _tctile(tc, [H, CH], BF16, name="pgbf")
        pgT = _tctile(tc, [128, TT, H], BF16, name="pgT")
        masks = [[_tctile(tc, [128, W], BF16, name=f"mask{i}_{s}") for s in range(7)]
                 for i in range(NQB)]
        padm = _tctile(tc, [H, CH], BF16, name="padm")
        iso = _tctile(tc, [128, 1], mybir.dt.uint8, name="iso")
        selL = _tctile(tc, [128, 4], F32, name="selL")
        selR = _tctile(tc, [128, 4], F32, name="selR")
        ident = _tctile(tc, [128, 128], BF16, name="ident")
        eexp = [_tctile(tc, [H, 128], BF16, name=f"eexp{o}") for o in range(KT)]
        onescol = _tctile(tc, [128, 1], BF16, name="onescol")
        eps_col = _tctile(tc, [1, 1], F32, name="eps_col")
        onesrow = _tctile(tc, [1, 128], BF16, name="onesrow")
        qg_own = _tctile(tc, [128, KT], BF16, name="qg_own")
        og_acc = _tctile(tc, [H, HID], F32, name="og_acc")
        og_part = _tctile(tc, [H, 65], F32, name="og_part")
        og4 = _tctile(tc, [H, 4, 65], F32, name="og4")
        og_tot = _tctile(tc, [H, 65], F32, name="og_tot")
        og_rcp = _tctile(tc, [H, 1], F32, name="og_rcp")
        og_bf = _tctile(tc, [H, D], BF16, name="og_bf")
        og_sb = _tctile(tc, [128, H], BF16, name="og_sb")
        mu_sb = _tctile(tc, [1, 512], F32, name="mu_sb")
        rstd_sb = _tctile(tc, [1, 512], F32, name="rstd_sb")
        m2_sb = _tctile(tc, [1, 512], F32, name="m2_sb")
        mu_bf = _tctile(tc, [1, 512], BF16, name="mu_bf")
        rstd_bf = _tctile(tc, [1, 512], BF16, name="rstd_bf")
        h1_bf = _tctile(tc, [128, 4], BF16, name="h1_bf")
        hb2_sb = _tctile(tc, [NCLS, 1], F32, name="hb2_sb")
        out_sb = _tctile(tc, [NCLS, 1], F32, name="out_sb")

        ctx = contextlib.ExitStack()
        wpool = ctx.enter_context(tc.tile_pool(name="wpool", bufs=6))
        kgpool = ctx.enter_context(tc.tile_pool(name="kgpool", bufs=2))
        vgpool = ctx.enter_context(tc.tile_pool(name="vgpool", bufs=2))
        ptpool = ctx.enter_context(tc.tile_pool(name="ptpool", bufs=4))
        pvspool = ctx.enter_context(tc.tile_pool(name="pvspool", bufs=3))
        gpool = ctx.enter_context(tc.tile_pool(name="gpool", bufs=6))
        fpool = ctx.enter_context(tc.tile_pool(name="fpool", bufs=6))
        tbfpool = ctx.enter_context(tc.tile_pool(name="tbfpool", bufs=2))
        ppool = ctx.enter_context(tc.tile_pool(name="ppool", bufs=4))
        hpool = ctx.enter_context(tc.tile_pool(name="hpool", bufs=2))
        ps_a = ctx.enter_context(tc.tile_pool(name="ps_a", bufs=2, space="PSUM"))
        ps_s = ctx.enter_context(tc.tile_pool(name="ps_s", bufs=2, space="PSUM"))
        ps_pv = ctx.enter_context(tc.tile_pool(name="ps_pv", bufs=2, space="PSUM"))
        ps_r = ctx.enter_context(tc.tile_pool(name="ps_r", bufs=2, space="PSUM"))

        # ---- static init ----
        for o in range(KT):
            nc.sync.dma_start(xT[o][:], x0T_d[o])
            nc.sync.dma_start(eexp[o][:], eexp_d[o])
        for i in range(NQB):
            for s in range(7):
                nc.sync.dma_start(masks[i][s][:], masks_d[i, s])
        nc.sync.dma_start(padm[:], padm_d[:])
        nc.sync.dma_start(iso[:], iso_d[:])
        nc.sync.dma_start(selL[:], selL_d[:])
        nc.sync.dma_start(selR[:], selR_d[:])
        nc.sync.dma_start(ident[:], ident_d[:])
        nc.vector.memset(onescol[:], 1.0)
        nc.vector.memset(eps_col[:], 1e-5)
        nc.vector.memset(onesrow[:], 1.0)
        for o in range(KT):
            nc.vector.memset(kwin[o][:, 12 * 128:], 0.0)
        nc.vector.memset(vwin[GSUB][:], 0.0)
        for t in range(13):
            nc.vector.memset(vwin[t][:, :, 64:65], 1.0)

        def wslab(dram, l, ki, cols=None, tag="w"):
            src = dram[l, ki * 128:(ki + 1) * 128]
            n = src.shape[-1]
            if cols is not None:
                src = src[:, cols[0]:cols[1]]
                n = cols[1] - cols[0]
            t = wpool.tile([128, n], BF16, tag=f"w{n}", name=f"{tag}{n}")
            nc.sync.dma_start(t[:], src)
            return t

        def pcol(vals_ap, n=KT):
            t = ppool.tile([128, n], F32, tag=f"pcol{n}", name=f"pcol{n}")
            nc.sync.dma_start(t[:], vals_ap.rearrange("(o p) -> p o", p=128))
            return t

        def layernorm(gcol, bcol):
            for n in range(2):
                sl = slice(n * 512, (n + 1) * 512)
                p_sum = ps_r.tile([1, 512], F32, tag="r", name="p_sum")
                p_sq = ps_r.tile([1, 512], F32, tag="r", name="p_sq")
                for o in range(KT):
                    tb = tbfpool.tile([128, 512], BF16, tag="tbf", name="tbf")
                    sq = tbfpool.tile([128, 512], BF16, tag="sqbf", name="sqbf")
                    nc.vector.tensor_copy(tb[:], xT[o][:, sl])
                    nc.vector.tensor_tensor(sq[:], xT[o][:, sl], xT[o][:, sl], ALU.mult)
                    nc.tensor.matmul(p_sum[:], lhsT=onescol[:], rhs=tb[:],
                                     start=(o == 0), stop=(o == KT - 1))
                    nc.tensor.matmul(p_sq[:], lhsT=onescol[:], rhs=sq[:],
                                     start=(o == 0), stop=(o == KT - 1))
                _act(mu_sb[:], p_sum[:], AF.Copy, scale=1.0 / HID)
                _act(m2_sb[:], p_sq[:], AF.Copy, scale=1.0 / HID)
                nc.vector.tensor_tensor(rstd_sb[:], mu_sb[:], mu_sb[:], ALU.mult)
                nc.vector.tensor_sub(rstd_sb[:], m2_sb[:], rstd_sb[:])
                _act(rstd_sb[:], rstd_sb[:], AF.Sqrt, bias=eps_col[:])
                nc.vector.reciprocal(rstd_sb[:], rstd_sb[:])
                nc.vector.tensor_copy(mu_bf[:], mu_sb[:])
                nc.vector.tensor_copy(rstd_bf[:], rstd_sb[:])
                mub = ps_a.tile([128, 512], F32, tag="a", name="mub")
                rb = ps_a.tile([128, 512], F32, tag="a", name="rb")
                nc.tensor.matmul(mub[:], lhsT=onesrow[:], rhs=mu_bf[:],
                                 start=True, stop=True)
                nc.tensor.matmul(rb[:], lhsT=onesrow[:], rhs=rstd_bf[:],
                                 start=True, stop=True)
                for o in range(KT):
                    nc.vector.tensor_sub(xT[o][:, sl], xT[o][:, sl], mub[:])
                    nc.vector.tensor_tensor(xT[o][:, sl], xT[o][:, sl], rb[:],
                                            ALU.mult)
                    _act(xT[o][:, sl], xT[o][:, sl], AF.Identity,
                         bias=bcol[:, o:o + 1], scale=gcol[:, o:o + 1])
                    nc.vector.tensor_copy(xbf[o][:, sl], xT[o][:, sl])

        def proj_T(wdram, l, out_tiles, out_col0, bias_col, tag):
            slabs = [wslab(wdram, l, ki, tag=tag) for ki in range(KT)]
            for o in range(KT):
                for n in range(2):
                    ps = ps_a.tile([128, 512], F32, tag="a", name="projp")
                    for ki in range(KT):
                        nc.tensor.matmul(
                            ps[:], lhsT=slabs[ki][:, o * 128:(o + 1) * 128],
                            rhs=xbf[ki][:, n * 512:(n + 1) * 512],
                            start=(ki == 0), stop=(ki == KT - 1))
                    c0 = out_col0 + n * 512
                    _act(out_tiles[o][:, c0:c0 + 512], ps[:], AF.Identity,
                         bias=bias_col[:, o:o + 1])

        def ag1r(l, ofs, shape, rank=None):
            src = ag1_in[l] if rank is None else ag1_out[l][rank]
            n = int(np.prod(shape))
            ap = src[ofs:ofs + n]
            if len(shape) == 2:
                return ap.rearrange("(p c) -> p c", c=shape[1])
            return ap.rearrange("(p h d) -> p h d", h=shape[1], d=shape[2])

        def halo_union(l, ofs, shape, out_ap, sel):
            for r in range(4):
                st = hpool.tile(list(shape), BF16, tag=f"hst{shape[-1]}",
                                name="hst")
                nc.sync.dma_start(st[:], ag1r(l, ofs, shape, rank=r))
                if r == 0:
                    nc.vector.tensor_scalar_mul(out_ap, st[:], sel[:, 0:1])
                else:
                    nc.vector.scalar_tensor_tensor(
                        out_ap, st[:], sel[:, r:r + 1], out_ap,
                        ALU.mult, ALU.add)

        # ================= embedding LN =================
        embg = pcol(embg_d[0])
        embb = pcol(embg_d[1])
        layernorm(embg, embb)

        # ================= layers =================
        for l in range(nl):
            bq_c = pcol(bcols_d[l, 0])
            bk_c = pcol(bcols_d[l, 1])
            bkg_c = pcol(bcols_d[l, 2])
            bqg_c = pcol(bcols_d[l, 3])
            bo_c = pcol(bcols_d[l, 4])

            # ---- K / Q projections (transposed) ----
            proj_T(wk_d, l, kwin, 2 * 128, bk_c, "wk")
            proj_T(wq_d, l, qT, 0, bq_c, "wq")

            # ---- V projection (token-major into vwin) ----
            bvb_t = ppool.tile([128, HID], BF16, tag="bvb", name="bvb")
            nc.sync.dma_start(bvb_t[:], bvb_d[l])
            vslabs = [wslab(wv_d, l, ki, tag="wv") for ki in range(KT)]
            for t in range(TT):
                for hf in range(2):
                    ps = ps_a.tile([128, 384], F32, tag="a", name="vproj")
                    for ki in range(KT):
                        nc.tensor.matmul(
                            ps[:], lhsT=xbf[ki][:, t * 128:(t + 1) * 128],
                            rhs=vslabs[ki][:, hf * 384:(hf + 1) * 384],
                            start=(ki == 0), stop=(ki == KT - 1))
                    nc.vector.tensor_tensor(
                        vwin[2 + t][:, hf * 6:(hf + 1) * 6, 0:64],
                        ps[:].rearrange("p (h d) -> p h d", h=6),
                        bvb_t[:, hf * 384:(hf + 1) * 384].rearrange(
                            "p (h d) -> p h d", h=6),
                        ALU.add)

            # ---- qg (from column 0) ----
            qgslabs = [wslab(wqg_d, l, ki, tag="wqg") for ki in range(KT)]
            for o in range(KT):
                ps = ps_a.tile([128, 1], F32, tag="a", name="qgp")
                for ki in range(KT):
                    nc.tensor.matmul(ps[:],
                                     lhsT=qgslabs[ki][:, o * 128:(o + 1) * 128],
                                     rhs=xbf[ki][:, 0:1],
                                     start=(ki == 0), stop=(ki == KT - 1))
                _act(qg_own[:, o:o + 1], ps[:], AF.Identity,
                     bias=bqg_c[:, o:o + 1])

            # ---- AG#1 ship ----
            for o in range(KT):
                nc.sync.dma_start(ag1r(l, _KF + o * 128 * W, [128, W]),
                                  kwin[o][:, 256:256 + W])
                nc.sync.dma_start(ag1r(l, _KL + o * 128 * W, [128, W]),
                                  kwin[o][:, 1024:1024 + W])
            for t in range(2):
                nc.sync.dma_start(ag1r(l, _VF + t * 128 * HID, [128, H, 64]),
                                  vwin[2 + t][:, :, 0:64])
                nc.sync.dma_start(ag1r(l, _VL + t * 128 * HID, [128, H, 64]),
                                  vwin[8 + t][:, :, 0:64])
            nc.sync.dma_start(ag1r(l, _QG, [128, KT]), qg_own[:])
            nc.gpsimd.collective_compute(
                "AllGather", ALU.bypass, replica_groups=RG,
                ins=[ag1_in[l][:]], outs=[ag1_out[l][:]])

            # ---- AG#1 consume ----
            for o in range(KT):
                halo_union(l, _KL + o * 128 * W, [128, W],
                           kwin[o][:, 0:W], selL)
                halo_union(l, _KF + o * 128 * W, [128, W],
                           kwin[o][:, 1536 - W:1536], selR)
                nc.sync.dma_start(
                    kwin[o][:, 1536:1537],
                    ag1r(l, _KF + o * 128 * W, [128, W], rank=0)[:, 0:1])
            for t in range(2):
                halo_union(l, _VL + t * 128 * HID, [128, H, 64],
                           vwin[t][:, :, 0:64], selL)
                halo_union(l, _VF + t * 128 * HID, [128, H, 64],
                           vwin[10 + t][:, :, 0:64], selR)
            nc.sync.dma_start(
                vwin[GSUB][0:1, :, 0:64],
                ag1r(l, _VF, [128, H, 64], rank=0)[0:1])
            nc.sync.dma_start(qg_own[:], ag1r(l, _QG, [128, KT], rank=0))

            # ---- kg projection + global scores sg ----
            kgslabs = [wslab(wkg_d, l, ki, tag="wkg") for ki in range(KT)]
            for o in range(KT):
                kg_t = kgpool.tile([128, CH], BF16, tag="kg", name="kg")
                for n in range(2):
                    ps = ps_a.tile([128, 512], F32, tag="a", name="kgp")
                    for ki in range(KT):
                        nc.tensor.matmul(
                            ps[:], lhsT=kgslabs[ki][:, o * 128:(o + 1) * 128],
                            rhs=xbf[ki][:, n * 512:(n + 1) * 512],
                            start=(ki == 0), stop=(ki == KT - 1))
                    _act(kg_t[:, n * 512:(n + 1) * 512], ps[:], AF.Identity,
                         bias=bkg_c[:, o:o + 1])
                for hh in range(2):      # heads 2o, 2o+1
                    h = 2 * o + hh
                    for n in range(2):
                        ps = ps_r.tile([1, 512], F32, tag="r", name="sgp")
                        nc.tensor.matmul(
                            ps[:],
                            lhsT=qg_own[hh * 64:(hh + 1) * 64, o:o + 1],
                            rhs=kg_t[hh * 64:(hh + 1) * 64,
                                     n * 512:(n + 1) * 512],
                            start=True, stop=True)
                        sgf = tbfpool.tile([1, 512], F32, tag="sgf",
                                           name="sgf")
                        _act(sgf[:], ps[:], AF.Copy)
                        nc.gpsimd.dma_start(
                            pgbf[h:h + 1, n * 512:(n + 1) * 512], sgf[:])

            # ---- pg = exp(sg) * padmask ; lsum ; pgT ----
            _act(pgbf[:], pgbf[:], AF.Exp)
            nc.vector.tensor_tensor(pgbf[:], pgbf[:], padm[:], ALU.mult)
            nc.vector.tensor_reduce(og_part[:, 64:65], pgbf[:],
                                    mybir.AxisListType.X, ALU.add)
            for t in range(TT):
                pst = ps_s.tile([128, H], BF16, tag="s", name="pgTp")
                nc.tensor.transpose(pst[:], pgbf[:, t * 128:(t + 1) * 128],
                                    ident[0:H, 0:H])
                nc.vector.tensor_copy(pgT[:, t, :], pst[:])

            # ---- vg projection + og partials ----
            bvgb_t = ppool.tile([128, HID], BF16, tag="bvb", name="bvgb")
            nc.sync.dma_start(bvgb_t[:], bvgb_d[l])
            vgslabs = [wslab(wvg_d, l, ki, tag="wvg") for ki in range(KT)]
            for t in range(TT):
                vg_t = vgpool.tile([128, HID], BF16, tag="vg", name="vg")
                for hf in range(2):
                    ps = ps_a.tile([128, 384], F32, tag="a", name="vgp")
                    for ki in range(KT):
                        nc.tensor.matmul(
                            ps[:], lhsT=xbf[ki][:, t * 128:(t + 1) * 128],
                            rhs=vgslabs[ki][:, hf * 384:(hf + 1) * 384],
                            start=(ki == 0), stop=(ki == KT - 1))
                    nc.vector.tensor_tensor(
                        vg_t[:, hf * 384:(hf + 1) * 384], ps[:],
                        bvgb_t[:, hf * 384:(hf + 1) * 384], ALU.add)
                for half, (c0, cn) in enumerate([(0, 512), (512, 256)]):
                    ps = ps_pv.tile([H, cn], F32, tag="pv", name="ogp")
                    nc.tensor.matmul(ps[:], lhsT=pgT[:, t, :],
                                     rhs=vg_t[:, c0:c0 + cn],
                                     start=True, stop=True)
                    if t == 0:
                        nc.vector.tensor_copy(og_acc[:, c0:c0 + cn], ps[:])
                    else:
                        nc.vector.tensor_add(og_acc[:, c0:c0 + cn],
                                             og_acc[:, c0:c0 + cn], ps[:])
            for h in range(H):
                nc.sync.dma_start(og_part[h:h + 1, 0:64],
                                  og_acc[h:h + 1, h * 64:(h + 1) * 64])

            # ---- AG#2: combine global softmax partials ----
            nc.sync.dma_start(ag2_in[l][:], og_part[:])
            nc.gpsimd.collective_compute(
                "AllGather", ALU.bypass, replica_groups=RG,
                ins=[ag2_in[l][:]], outs=[ag2_out[l][:]])
            for r in range(4):
                nc.sync.dma_start(og4[:, r, :], ag2_out[l][r])
            nc.vector.tensor_add(og_tot[:], og4[:, 0, :], og4[:, 1, :])
            nc.vector.tensor_add(og_tot[:], og_tot[:], og4[:, 2, :])
            nc.vector.tensor_add(og_tot[:], og_tot[:], og4[:, 3, :])
            nc.vector.reciprocal(og_rcp[:], og_tot[:, 64:65])
            nc.vector.tensor_scalar_mul(og_bf[:], og_tot[:, 0:64], og_rcp[:])
            pso = ps_s.tile([64, H], BF16, tag="s", name="ogT")
            nc.tensor.transpose(pso[:], og_bf[:], ident[0:H, 0:H])
            og_tmp = tbfpool.tile([64, H], BF16, tag="ogtmp", name="og_tmp")
            nc.vector.tensor_copy(og_tmp[:], pso[:])
            nc.sync.dma_start(og_sb[0:64, :], og_tmp[:])
            nc.sync.dma_start(og_sb[64:128, :], og_tmp[:])

            # ---- banded attention ----
            for i in range(NQB):
                qsl = slice(i * W, (i + 1) * W)
                for h in range(H):
                    o, p0 = h // 2, (h % 2) * 64
                    pv = ps_pv.tile([65, W], F32, tag="pv", name="pv")
                    for s in range(7):
                        wsub = 2 * i + s if s < 6 else GSUB
                        ps = ps_s.tile([128, W], F32, tag="s", name="sp")
                        nc.tensor.matmul(
                            ps[:],
                            lhsT=kwin[o][p0:p0 + 64,
                                         wsub * 128:(wsub + 1) * 128],
                            rhs=qT[o][p0:p0 + 64, qsl],
                            start=True, stop=True)
                        pt = ptpool.tile([128, W], BF16, tag="pt", name="pt")
                        _act(pt[:], ps[:], AF.Exp)
                        nc.vector.tensor_tensor(pt[:], pt[:], masks[i][s][:],
                                                ALU.mult)
                        nc.tensor.matmul(pv[:], lhsT=vwin[wsub][:, h, :],
                                         rhs=pt[:],
                                         start=(s == 0), stop=(s == 6))
                    pvs = pvspool.tile([65, W], F32, tag="pvs", name="pvs")
                    _act(pvs[:], pv[:], AF.Copy)
                    nc.gpsimd.dma_start(attnT[o][p0:p0 + 64, qsl],
                                        pvs[0:64, :])
                    nc.sync.dma_start(sums[h:h + 1, qsl], pvs[64:65, :])

            # ---- normalize + og select ----
            nc.vector.reciprocal(sums[:], sums[:])
            nc.vector.tensor_copy(rsums_bf[:], sums[:])
            for o in range(KT):
                for n in range(2):
                    psd = ps_a.tile([128, 512], F32, tag="a", name="divp")
                    nc.tensor.matmul(psd[:], lhsT=eexp[o][:],
                                     rhs=rsums_bf[:, n * 512:(n + 1) * 512],
                                     start=True, stop=True)
                    nc.vector.tensor_tensor(
                        attnT[o][:, n * 512:(n + 1) * 512],
                        attnT[o][:, n * 512:(n + 1) * 512], psd[:], ALU.mult)
            for h in range(H):
                o, p0 = h // 2, (h % 2) * 64
                nc.vector.select(attnT[o][p0:p0 + 64, 0:1],
                                 iso[p0:p0 + 64, :],
                                 og_sb[p0:p0 + 64, h:h + 1],
                                 attnT[o][p0:p0 + 64, 0:1])

            # ---- wo + residual ----
            woslabs = [wslab(wo_d, l, ki, tag="wo") for ki in range(KT)]
            for o in range(KT):
                for n in range(2):
                    ps = ps_a.tile([128, 512], F32, tag="a", name="wop")
                    for ki in range(KT):
                        nc.tensor.matmul(
                            ps[:], lhsT=woslabs[ki][:, o * 128:(o + 1) * 128],
                            rhs=attnT[ki][:, n * 512:(n + 1) * 512],
                            start=(ki == 0), stop=(ki == KT - 1))
                    _act(ps[:], ps[:], AF.Identity, bias=bo_c[:, o:o + 1])
                    nc.vector.tensor_add(xT[o][:, n * 512:(n + 1) * 512],
                                         xT[o][:, n * 512:(n + 1) * 512],
                                         ps[:])

            if DEBUG_DUMPS:
                for o in range(KT):
                    nc.sync.dma_start(dbg_d[l, 0, o], xT[o][:])
            # ---- LN1 ----
            ln1g_c = pcol(bcols_d[l, 5])
            ln1b_c = pcol(bcols_d[l, 6])
            layernorm(ln1g_c, ln1b_c)

            # ---- FFN ----
            b1_c = pcol(b1_d[l], n=FFN // 128)
            b2_c = pcol(bcols_d[l, 9])
            for n in range(2):
                sl = slice(n * 512, (n + 1) * 512)
                faccs = [fpool.tile([128, 512], F32, tag="facc", name="facc")
                         for _ in range(KT)]
                for c in range(FFN // 512):
                    w1c = [wslab(w1_d, l, ki, cols=(c * 512, (c + 1) * 512),
                                 tag="w1") for ki in range(KT)]
                    g_ts = []
                    for oo in range(4):
                        psg = ps_a.tile([128, 512], F32, tag="a", name="w1p")
                        for ki in range(KT):
                            nc.tensor.matmul(
                                psg[:],
                                lhsT=w1c[ki][:, oo * 128:(oo + 1) * 128],
                                rhs=xbf[ki][:, sl],
                                start=(ki == 0), stop=(ki == KT - 1))
                        g_t = gpool.tile([128, 512], BF16, tag="g", name="g")
                        _act(g_t[:], psg[:], AF.Gelu,
                             bias=b1_c[:, c * 4 + oo:c * 4 + oo + 1])
                        g_ts.append(g_t)
                    w2c = [wslab(w2_d, l, c * 4 + cc, tag="w2")
                           for cc in range(4)]
                    for o in range(KT):
                        psf = ps_a.tile([128, 512], F32, tag="a", name="w2p")
                        for cc in range(4):
                            nc.tensor.matmul(
                                psf[:],
                                lhsT=w2c[cc][:, o * 128:(o + 1) * 128],
                                rhs=g_ts[cc][:],
                                start=(cc == 0), stop=(cc == 3))
                        if c == 0:
                            nc.vector.tensor_copy(faccs[o][:], psf[:])
                        else:
                            nc.vector.tensor_add(faccs[o][:], faccs[o][:],
                                                 psf[:])
                for o in range(KT):
                    _act(faccs[o][:], faccs[o][:], AF.Identity,
                         bias=b2_c[:, o:o + 1])
                    nc.vector.tensor_add(xT[o][:, sl], xT[o][:, sl],
                                         faccs[o][:])

            # ---- LN2 ----
            ln2g_c = pcol(bcols_d[l, 7])
            ln2b_c = pcol(bcols_d[l, 8])
            layernorm(ln2g_c, ln2b_c)
            if DEBUG_DUMPS:
                for o in range(KT):
                    nc.sync.dma_start(dbg_d[l, 1, o], xT[o][:])

        # ================= outputs =================
        for o in range(KT):
            nc.sync.dma_start(xfin_d[o], xT[o][:])

        # classifier head on column 0
        hb1_c = pcol(hb1_d[:], n=4)
        nc.sync.dma_start(hb2_sb[:], hb2_d[:].rearrange("(p o) -> p o", o=1))
        h1slabs = [wslab(hw1_d, 0, ki, tag="hw1")
                   for ki in range(KT)] if False else None
        h1s = []
        for ki in range(KT):
            t = wpool.tile([128, 512], BF16, tag="w512", name="hw1s")
            nc.sync.dma_start(t[:], hw1_d[ki * 128:(ki + 1) * 128])
            h1s.append(t)
        for o4 in range(4):
            ps = ps_a.tile([128, 1], F32, tag="a", name="h1p")
            for ki in range(KT):
                nc.tensor.matmul(ps[:], lhsT=h1s[ki][:, o4 * 128:(o4 + 1) * 128],
                                 rhs=xbf[ki][:, 0:1],
                                 start=(ki == 0), stop=(ki == KT - 1))
            _act(h1_bf[:, o4:o4 + 1], ps[:], AF.Relu, bias=hb1_c[:, o4:o4 + 1])
        h2s = []
        for ki in range(4):
            t = wpool.tile([128, NCLS], BF16, tag="w43", name="hw2s")
            nc.sync.dma_start(t[:], hw2_d[ki * 128:(ki + 1) * 128])
            h2s.append(t)
        ps2 = ps_a.tile([NCLS, 1], F32, tag="a", name="h2p")
        for ki in range(4):
            nc.tensor.matmul(ps2[:], lhsT=h2s[ki][:], rhs=h1_bf[:, ki:ki + 1],
                             start=(ki == 0), stop=(ki == 3))
        _act(out_sb[:], ps2[:], AF.Identity, bias=hb2_sb[:])
        nc.sync.dma_start(logits_d[:], out_sb[:])

        ctx.close()
        _pers_ctx.close()

    nc.finalize()
    return nc


# ======================= host side =======================

def _build_masks(j, att_row):
    """[NQB, 7, 128, W] multiplicative masks for one core (chunk j)."""
    base = CH * j
    out = np.zeros((NQB, 7, 128, W), dtype=np.float32)
    qq = np.arange(W)
    for i in range(NQB):
        qpos = base + W * i + qq          # [W]
        for s in range(6):
            r = np.arange(128)
            kpos = base + W * i + 128 * (s - 2) + r   # [128]
            kk, qq2 = np.meshgrid(kpos, qpos, indexing="ij")
            valid = (np.abs(kk - qq2) <= W) & (kk >= 0) & (kk < S) & (kk != 0)
            kc = np.clip(kpos, 0, S - 1)
            valid &= (att_row[kc] > 0)[:, None]
            out[i, s] = valid
        out[i, 6, 0, :] = 1.0 * (att_row[0] > 0)
    return out.astype(BF)


_NC_CACHE = {}


def _get_program():
    if "nc" not in _NC_CACHE:
        _NC_CACHE["nc"] = build_program()
    return _NC_CACHE["nc"]


def kernel(input_ids, attention_mask, word_emb, pos_emb, emb_ln_g, emb_ln_b,
           layers, head_w1, head_b1, head_w2, head_b2):
    nl = NLAYERS
    input_ids = np.asarray(input_ids)
    attention_mask = np.asarray(attention_mask)
    f32 = lambda a: np.ascontiguousarray(np.asarray(a), dtype=np.float32)
    bf = lambda a: np.ascontiguousarray(np.asarray(a)).astype(BF)
    ly = {k: np.asarray(v) for k, v in layers.items()}

    scale = 1.0 / np.sqrt(D)
    wq = bf(ly["wq"][:nl] * scale)
    wqg = bf(ly["wqg"][:nl] * scale)
    bcols = np.stack([
        f32(ly["bq"][:nl] * scale), f32(ly["bk"][:nl]), f32(ly["bkg"][:nl]),
        f32(ly["bqg"][:nl] * scale), f32(ly["bo"][:nl]), f32(ly["ln1g"][:nl]),
        f32(ly["ln1b"][:nl]), f32(ly["ln2g"][:nl]), f32(ly["ln2b"][:nl]),
        f32(ly["b2"][:nl])], axis=1)
    bvb = np.broadcast_to(bf(ly["bv"][:nl])[:, None, :], (nl, 128, HID))
    bvgb = np.broadcast_to(bf(ly["bvg"][:nl])[:, None, :], (nl, 128, HID))

    eexp = np.zeros((KT, H, 128), dtype=np.float32)
    for o in range(KT):
        for p in range(128):
            eexp[o, (o * 128 + p) // 64, p] = 1.0

    shared = {
        "wq": wq, "wk": bf(ly["wk"][:nl]), "wv": bf(ly["wv"][:nl]),
        "wkg": bf(ly["wkg"][:nl]), "wvg": bf(ly["wvg"][:nl]), "wqg": wqg,
        "wo": bf(ly["wo"][:nl]), "w1": bf(ly["w1"][:nl]), "w2": bf(ly["w2"][:nl]),
        "bcols": np.ascontiguousarray(bcols),
        "b1": f32(ly["b1"][:nl]),
        "bvb": np.ascontiguousarray(bvb), "bvgb": np.ascontiguousarray(bvgb),
        "embln": np.stack([f32(emb_ln_g), f32(emb_ln_b)]),
        "ident": np.eye(128, dtype=np.float32).astype(BF),
        "eexp": eexp.astype(BF),
        "hw1": bf(head_w1), "hb1": f32(head_b1),
        "hw2": bf(head_w2), "hb2": f32(head_b2),
    }

    emb = np.asarray(word_emb)[input_ids] + np.asarray(pos_emb)[None]
    emb = np.ascontiguousarray(emb, dtype=np.float32)   # [B, S, HID]

    in_maps = []
    for c in range(8):
        b, j = c // 4, c % 4
        x0T = emb[b, CH * j:CH * (j + 1)].T.reshape(KT, 128, CH)
        att = np.asarray(attention_mask[b])
        padm = np.broadcast_to((att[CH * j:CH * (j + 1)] > 0)[None, :], (H, CH))
        selL = np.zeros((128, 4), np.float32)
        selR = np.zeros((128, 4), np.float32)
        if j > 0:
            selL[:, j - 1] = 1.0
        if j < 3:
            selR[:, j + 1] = 1.0
        iso = np.full((128, 1), 1.0 if j == 0 else 0.0, np.float32)
        m = dict(shared)
        m["x0T"] = np.ascontiguousarray(x0T)
        m["masks"] = _build_masks(j, att)
        m["padm"] = np.ascontiguousarray(padm.astype(BF))
        m["iso"] = iso.astype(np.uint8)
        m["selL"] = selL
        m["selR"] = selR
        in_maps.append(m)

    nc = _get_program()
    trace = bool(int(os.environ.get("KERNEL_TRACE", "0")))
    res = run_bass_kernel_spmd(nc, in_maps, list(range(8)), trace=trace)
    out = np.stack([res.results[0]["logits"][:, 0],
                    res.results[4]["logits"][:, 0]])
    _NC_CACHE["last_results"] = res
    return out.astype(np.float32)
